# revision 2
# baseline (speedup 1.0000x reference)
"""Distributed Trainium2 kernel for the dense-graph GNN layer.

Math: with xn = x/||x|| (rows), G = xn@xn.T, d = rsqrt(G@1),
out = (diag(d) G diag(d) x) W.  The N x N Gram matrix is never needed:
  G @ 1        = xn @ t,            t = colsum(xn)            [D]
  diag(d) G diag(d) x = f * (x @ z),  z = x.T @ diag(f) @ x   [D, D]
  f_i = d_i / ||x_i||   (combines both scalings; z is symmetric)
  out = f * (x @ (z @ W))

Collective-free layout: collectives on this part pay a ~10-45us entry
barrier + launch-skew penalty that dwarfs the actual math, so instead
every core redundantly computes the global reductions (t and z) from the
FULL x (4 MB as bf16, ~11us HBM stream) and writes only its own 1024-row
output slice.  Each core receives x rolled so that its own rows come
first; the program itself is rank-free (identical SPMD program, no
cross-core traffic at all).
"""

import os
import sys

import numpy as np
from ml_dtypes import bfloat16

for _p in ("/opt/trn_rl_repo", "/root/.axon_site/_ro/trn_rl_repo"):
    if os.path.isdir(_p) and _p not in sys.path:
        sys.path.insert(0, _p)

import concourse.bacc as bacc
import concourse.mybir as mybir
import concourse.tile as tile
import concourse.masks as masks
from concourse import bass_utils
from concourse.bass_types import AP as _AP

R = 8                 # cores
N, D = 8192, 256
NL = N // R           # 1024 rows owned per core
P = 128
NT = N // P           # 64 row tiles of the full x
LT = NL // P          # 8 local row tiles
BLK = 8               # tiles per DMA block
NB = NT // BLK        # 8 blocks
F32 = mybir.dt.float32
BF16 = mybir.dt.bfloat16
AF = mybir.ActivationFunctionType
ALU = mybir.AluOpType

_cache = {}


def _program(tc, x, W, out):
    nc = tc.nc
    with (
        tc.tile_pool(name="persist", bufs=1) as pp,
        tc.tile_pool(name="work", bufs=3) as wp,
        tc.tile_pool(name="psum", bufs=1, space="PSUM") as psp,
        tc.tile_pool(name="psumw", bufs=4, space="PSUM") as psw,
    ):
        xb = pp.tile([P, NT * D], BF16)    # full x, tile i at [:, i*D:(i+1)*D]
        xT = pp.tile([P, 2 * NL], BF16)    # local-shard x.T, chunk c at c*NL+i*P
        W_sb = pp.tile([P, 2 * D], F32)
        Wb = pp.tile([P, 2 * D], BF16)
        identb = pp.tile([P, P], BF16)
        ones1b = pp.tile([1, P], BF16)
        t_sbb = pp.tile([1, D], BF16)
        tb = pp.tile([P, D], BF16)         # t broadcast to 128 partitions
        zb = pp.tile([P, 2 * D], BF16)     # z chunk c = z[c*128:(c+1)*128, :]
        zwb = pp.tile([P, 2 * D], BF16)    # z@W chunk c likewise

        ss = pp.tile([P, NT], F32)         # row sumsq
        nrm = pp.tile([P, NT], F32)
        invn = pp.tile([P, NT], F32)
        invnb = pp.tile([P, NT], BF16)
        stl = pp.tile([P, NT], F32)        # rowsum(x * t)
        s_t = pp.tile([P, NT], F32)
        sq_s = pp.tile([P, NT], F32)
        dd = pp.tile([P, NT], F32)
        f_t = pp.tile([P, NT], F32)

        masks.make_identity(nc, identb[:])
        nc.gpsimd.memset(ones1b[:], 1.0)
        for kc in range(2):
            nc.sync.dma_start(W_sb[:, kc * D:(kc + 1) * D], W[kc * P:(kc + 1) * P, :])
        nc.vector.tensor_copy(Wb[:], W_sb[:])

        psum_t = psp.tile([1, D], F32, name="psum_t")
        psum_z0 = psp.tile([P, D], F32, name="pz0")
        psum_z1 = psp.tile([P, D], F32, name="pz1")

        # ---- stream phase: load full x, row sumsq, colsum(xn) via matmul ----
        for b in range(NB):
            dst = xb[:, b * BLK * D:(b + 1) * BLK * D].rearrange("p (t d) -> p t d", t=BLK)
            src = x[b * BLK * P:(b + 1) * BLK * P, :].rearrange("(t p) d -> p t d", t=BLK)
            nc.sync.dma_start(dst, src)

            if b == 0:
                # transpose the local shard (tiles 0..7 of the rolled x);
                # queued first on PE so it overlaps later blocks' DMA
                for i in range(LT):
                    for c in range(2):
                        pt = psw.tile([P, P], BF16, tag="pw", name=f"pt{i}_{c}")
                        nc.tensor.transpose(
                            pt[:], xb[:, i * D + c * P:i * D + (c + 1) * P], identb[:]
                        )
                        nc.vector.tensor_copy(
                            xT[:, c * NL + i * P:c * NL + (i + 1) * P], pt[:]
                        )

            for j in range(BLK):
                i = b * BLK + j
                scr = wp.tile([P, D], BF16, tag="sq", name=f"sq{i}")
                nc.scalar.activation(scr[:], xb[:, i * D:(i + 1) * D], AF.Square,
                                     accum_out=ss[:, i:i + 1])
            blk = slice(b * BLK, (b + 1) * BLK)
            nc.scalar.activation(nrm[:, blk], ss[:, blk], AF.Sqrt)
            nc.vector.reciprocal(invn[:, blk], nrm[:, blk])
            nc.vector.tensor_copy(invnb[:, blk], invn[:, blk])
            for j in range(BLK):
                i = b * BLK + j
                nc.tensor.matmul(
                    psum_t[:], lhsT=invnb[:, i:i + 1], rhs=xb[:, i * D:(i + 1) * D],
                    start=(i == 0), stop=(i == NT - 1),
                )

        # ---- t -> broadcast to all partitions (1-partition matmul trick) ----
        nc.vector.tensor_copy(t_sbb[:], psum_t[:])
        psum_tb = psw.tile([P, D], F32, tag="pw", name="ptb")
        nc.tensor.matmul(psum_tb[:], lhsT=ones1b[:], rhs=t_sbb[:], start=True, stop=True)
        nc.vector.tensor_copy(tb[:], psum_tb[:])

        # ---- z phase: degrees, f, g = f*x, z = x.T @ diag(f) @ x ----
        tb_ap = tb[:]
        for b in range(NB):
            blk = slice(b * BLK, (b + 1) * BLK)
            s3 = wp.tile([P, BLK * D], BF16, tag="s3", name=f"s3_{b}")
            s3v = s3[:].rearrange("p (t d) -> p t d", t=BLK)
            xb3 = xb[:, b * BLK * D:(b + 1) * BLK * D].rearrange("p (t d) -> p t d", t=BLK)
            t_rep = _AP(tb_ap.tensor, tb_ap.offset, [tb_ap.ap[0], [0, BLK], tb_ap.ap[1]])
            nc.vector.tensor_mul(s3v, xb3, t_rep)
            nc.vector.tensor_reduce(stl[:, blk], s3v, axis=mybir.AxisListType.X, op=ALU.add)
            nc.vector.tensor_mul(s_t[:, blk], stl[:, blk], invn[:, blk])
            nc.scalar.activation(sq_s[:, blk], s_t[:, blk], AF.Sqrt)
            nc.vector.reciprocal(dd[:, blk], sq_s[:, blk])
            nc.vector.tensor_mul(f_t[:, blk], dd[:, blk], invn[:, blk])
            for j in range(BLK):
                i = b * BLK + j
                g = wp.tile([P, D], BF16, tag="g", name=f"g{i}")
                if j % 2 == 0:
                    nc.scalar.mul(g[:], xb[:, i * D:(i + 1) * D], f_t[:, i:i + 1])
                else:
                    nc.gpsimd.tensor_scalar_mul(g[:], xb[:, i * D:(i + 1) * D],
                                                f_t[:, i:i + 1])
                for c, pz in ((0, psum_z0), (1, psum_z1)):
                    nc.tensor.matmul(
                        pz[:], lhsT=xb[:, i * D + c * P:i * D + (c + 1) * P], rhs=g[:],
                        start=(i == 0), stop=(i == NT - 1),
                    )

        # ---- zw = z @ W (z is symmetric, so z chunks serve as lhsT) ----
        nc.vector.tensor_copy(zb[:, 0:D], psum_z0[:])
        nc.vector.tensor_copy(zb[:, D:2 * D], psum_z1[:])
        for m in range(2):
            pzw = psw.tile([P, D], F32, tag="pw", name=f"pzw{m}")
            for kc in range(2):
                nc.tensor.matmul(
                    pzw[:], lhsT=zb[:, kc * D + m * P:kc * D + (m + 1) * P],
                    rhs=Wb[:, kc * D:(kc + 1) * D],
                    start=(kc == 0), stop=(kc == 1),
                )
            nc.vector.tensor_copy(zwb[:, m * D:(m + 1) * D], pzw[:])

        # ---- phase C: out = f * (x_local @ zw) ----
        for i in range(LT):
            po = psw.tile([P, D], F32, tag="pw", name=f"po{i}")
            for c in range(2):
                nc.tensor.matmul(
                    po[:], lhsT=xT[:, c * NL + i * P:c * NL + (i + 1) * P],
                    rhs=zwb[:, c * D:(c + 1) * D],
                    start=(c == 0), stop=(c == 1),
                )
            o_sb = wp.tile([P, D], F32, tag="o", name=f"o{i}")
            nc.scalar.mul(o_sb[:], po[:], f_t[:, i:i + 1])
            nc.sync.dma_start(out[i * P:(i + 1) * P, :], o_sb[:])


def _build():
    nc = bacc.Bacc("TRN2", target_bir_lowering=False, debug=False, num_devices=R)
    x = nc.dram_tensor("x", [N, D], BF16, kind="ExternalInput")
    W = nc.dram_tensor("W", [D, D], F32, kind="ExternalInput")
    out = nc.dram_tensor("out", [NL, D], F32, kind="ExternalOutput")
    with tile.TileContext(nc) as tc:
        _program(tc, x.ap() if hasattr(x, "ap") else x, W.ap() if hasattr(W, "ap") else W, out.ap() if hasattr(out, "ap") else out)
    nc.finalize()
    return nc


def _run(inputs, trace=False):
    if "nc" not in _cache:
        _cache["nc"] = _build()
    nc = _cache["nc"]
    x = np.asarray(inputs["x"], dtype=np.float32)
    W = np.ascontiguousarray(inputs["W"], dtype=np.float32)
    xb16 = x.astype(bfloat16)
    in_maps = []
    for r in range(R):
        # roll so core r's own 1024 rows come first -> rank-free program
        xr = np.ascontiguousarray(
            np.concatenate([xb16[r * NL:], xb16[:r * NL]], axis=0))
        in_maps.append({"x": xr, "W": W})
    res = bass_utils.run_bass_kernel_spmd(
        nc, in_maps, core_ids=list(range(R)), trace=trace,
    )
    out = np.concatenate([res.results[r]["out"] for r in range(R)], axis=0)
    return out, res


def kernel(**inputs) -> np.ndarray:
    out, _ = _run(inputs, trace=False)
    return out


# revision 3
# speedup vs baseline: 2.1296x; 2.1296x over previous
"""Distributed Trainium2 kernel for the dense-graph GNN layer.

Math: with xn = x/||x|| (rows), G = xn@xn.T, d = rsqrt(G@1),
out = (diag(d) G diag(d) x) W.  The N x N Gram matrix is never needed:
  G @ 1        = xn @ t,            t = colsum(xn)            [D]
  diag(d) G diag(d) x = f * (x @ z),  z = x.T @ diag(f) @ x   [D, D]
  f_i = d_i / ||x_i||   (combines both scalings; z is symmetric)
  out = f * (x @ (z @ W))

Collective-free layout: collectives on this part pay a ~10-45us entry
barrier + launch-skew penalty that dwarfs the actual math, so instead
every core redundantly computes the global reductions (t and z) from the
FULL x (4 MB as bf16, ~11us HBM stream) and writes only its own 1024-row
output slice.  Each core receives x rolled so that its own rows come
first (the program is rank-free), plus its own shard pre-transposed so
no PE transposes are needed.  Bulk elementwise runs on DVE as large 3D
broadcast-AP ops; row-sum reduces are split DVE/Act to balance engines.
"""

import os
import sys

import numpy as np
from ml_dtypes import bfloat16

for _p in ("/opt/trn_rl_repo", "/root/.axon_site/_ro/trn_rl_repo"):
    if os.path.isdir(_p) and _p not in sys.path:
        sys.path.insert(0, _p)

import concourse.bacc as bacc
import concourse.mybir as mybir
import concourse.tile as tile
from concourse import bass_utils
from concourse.bass_types import AP as _AP

R = 8                 # cores
N, D = 8192, 256
NL = N // R           # 1024 rows owned per core
P = 128
NT = N // P           # 64 row tiles of the full x
LT = NL // P          # 8 local row tiles
BLK = 8               # tiles per DMA block
NB = NT // BLK        # 8 blocks
F32 = mybir.dt.float32
BF16 = mybir.dt.bfloat16
AF = mybir.ActivationFunctionType
ALU = mybir.AluOpType

# blocks whose row-sum reduces run on the Act (scalar) engine instead of DVE
SCALAR_BLOCKS = (1, 3, 5)

_cache = {}


def _bcast_tile_ap(src_ap, ntile):
    """[P, D] AP -> [P, ntile, D] with the tile axis broadcast (stride 0)."""
    return _AP(src_ap.tensor, src_ap.offset,
               [src_ap.ap[0], [0, ntile], src_ap.ap[1]])


def _bcast_col_ap(src_ap, width):
    """[P, ntile] AP -> [P, ntile, width] broadcasting each column (stride 0)."""
    return _AP(src_ap.tensor, src_ap.offset,
               [src_ap.ap[0], src_ap.ap[1], [0, width]])


def _program(tc, x, xTl, W, out):
    nc = tc.nc
    with (
        tc.tile_pool(name="persist", bufs=1) as pp,
        tc.tile_pool(name="work", bufs=3) as wp,
        tc.tile_pool(name="psum", bufs=1, space="PSUM") as psp,
        tc.tile_pool(name="psumw", bufs=4, space="PSUM") as psw,
    ):
        xb = pp.tile([P, NT * D], BF16)    # full x, tile i at [:, i*D:(i+1)*D]
        xT = pp.tile([P, 2 * NL], BF16)    # local-shard x.T, chunk c at c*NL
        W_sb = pp.tile([P, 2 * D], F32)
        Wb = pp.tile([P, 2 * D], BF16)
        ones1b = pp.tile([1, P], BF16)
        t_sbb = pp.tile([1, D], BF16)
        tb = pp.tile([P, D], BF16)         # t broadcast to 128 partitions
        zb = pp.tile([P, 2 * D], BF16)     # z chunk c = z[c*128:(c+1)*128, :]
        zwb = pp.tile([P, 2 * D], BF16)    # z@W chunk c likewise

        ss = pp.tile([P, NT], F32)         # row sumsq
        nrm = pp.tile([P, NT], F32)
        invn = pp.tile([P, NT], F32)
        invnb = pp.tile([P, NT], BF16)
        stl = pp.tile([P, NT], F32)        # rowsum(x * t)
        s_t = pp.tile([P, NT], F32)
        sq_s = pp.tile([P, NT], F32)
        dd = pp.tile([P, NT], F32)
        f_t = pp.tile([P, NT], F32)

        nc.gpsimd.memset(ones1b[:], 1.0)
        for kc in range(2):
            nc.sync.dma_start(W_sb[:, kc * D:(kc + 1) * D], W[kc * P:(kc + 1) * P, :])
        nc.vector.tensor_copy(Wb[:], W_sb[:])
        for c in range(2):
            nc.sync.dma_start(xT[:, c * NL:(c + 1) * NL], xTl[c * P:(c + 1) * P, :])

        psum_t = psp.tile([1, D], F32, name="psum_t")
        psum_z0 = psp.tile([P, D], F32, name="pz0")
        psum_z1 = psp.tile([P, D], F32, name="pz1")

        # ---- stream phase: load full x, row sumsq, colsum(xn) via matmul ----
        for b in range(NB):
            dst = xb[:, b * BLK * D:(b + 1) * BLK * D].rearrange("p (t d) -> p t d", t=BLK)
            src = x[b * BLK * P:(b + 1) * BLK * P, :].rearrange("(t p) d -> p t d", t=BLK)
            nc.sync.dma_start(dst, src)

            blk = slice(b * BLK, (b + 1) * BLK)
            xb3 = xb[:, b * BLK * D:(b + 1) * BLK * D].rearrange("p (t d) -> p t d", t=BLK)
            if b in SCALAR_BLOCKS:
                for j in range(BLK):
                    i = b * BLK + j
                    scr = wp.tile([P, D], BF16, tag="sq", name=f"sq{i}")
                    nc.scalar.activation(scr[:], xb[:, i * D:(i + 1) * D], AF.Square,
                                         accum_out=ss[:, i:i + 1])
            else:
                s3 = wp.tile([P, BLK * D], BF16, tag="s3", name=f"sq3_{b}")
                s3v = s3[:].rearrange("p (t d) -> p t d", t=BLK)
                nc.vector.tensor_mul(s3v, xb3, xb3)
                nc.vector.tensor_reduce(ss[:, blk], s3v, axis=mybir.AxisListType.X, op=ALU.add)
            nc.scalar.activation(nrm[:, blk], ss[:, blk], AF.Sqrt)
            nc.vector.reciprocal(invn[:, blk], nrm[:, blk])
            nc.vector.tensor_copy(invnb[:, blk], invn[:, blk])
            for j in range(BLK):
                i = b * BLK + j
                nc.tensor.matmul(
                    psum_t[:], lhsT=invnb[:, i:i + 1], rhs=xb[:, i * D:(i + 1) * D],
                    start=(i == 0), stop=(i == NT - 1),
                )

        # ---- t -> broadcast to all partitions (1-partition matmul trick) ----
        nc.vector.tensor_copy(t_sbb[:], psum_t[:])
        psum_tb = psw.tile([P, D], F32, tag="pw", name="ptb")
        nc.tensor.matmul(psum_tb[:], lhsT=ones1b[:], rhs=t_sbb[:], start=True, stop=True)
        nc.vector.tensor_copy(tb[:], psum_tb[:])

        # ---- z phase: degrees, f, g = f*x, z = x.T @ diag(f) @ x ----
        tb_ap = tb[:]
        for b in range(NB):
            blk = slice(b * BLK, (b + 1) * BLK)
            xb3 = xb[:, b * BLK * D:(b + 1) * BLK * D].rearrange("p (t d) -> p t d", t=BLK)
            t_rep = _bcast_tile_ap(tb_ap, BLK)
            s3 = wp.tile([P, BLK * D], BF16, tag="s3", name=f"s3_{b}")
            s3v = s3[:].rearrange("p (t d) -> p t d", t=BLK)
            nc.vector.tensor_mul(s3v, xb3, t_rep)
            if b in SCALAR_BLOCKS:
                for j in range(BLK):
                    i = b * BLK + j
                    scr = wp.tile([P, D], BF16, tag="sq", name=f"cp{i}")
                    nc.scalar.activation(scr[:], s3[:, j * D:(j + 1) * D], AF.Copy,
                                         accum_out=stl[:, i:i + 1])
            else:
                nc.vector.tensor_reduce(stl[:, blk], s3v, axis=mybir.AxisListType.X, op=ALU.add)
            nc.vector.tensor_mul(s_t[:, blk], stl[:, blk], invn[:, blk])
            nc.scalar.activation(sq_s[:, blk], s_t[:, blk], AF.Sqrt)
            nc.vector.reciprocal(dd[:, blk], sq_s[:, blk])
            nc.vector.tensor_mul(f_t[:, blk], dd[:, blk], invn[:, blk])

            g3 = wp.tile([P, BLK * D], BF16, tag="g", name=f"g{b}")
            g3v = g3[:].rearrange("p (t d) -> p t d", t=BLK)
            f_rep = _bcast_col_ap(f_t[:, blk], D)
            nc.vector.tensor_mul(g3v, xb3, f_rep)
            for j in range(BLK):
                i = b * BLK + j
                for c, pz in ((0, psum_z0), (1, psum_z1)):
                    nc.tensor.matmul(
                        pz[:], lhsT=xb[:, i * D + c * P:i * D + (c + 1) * P],
                        rhs=g3[:, j * D:(j + 1) * D],
                        start=(i == 0), stop=(i == NT - 1),
                    )

        # ---- zw = z @ W (z is symmetric, so z chunks serve as lhsT) ----
        nc.vector.tensor_copy(zb[:, 0:D], psum_z0[:])
        nc.vector.tensor_copy(zb[:, D:2 * D], psum_z1[:])
        for m in range(2):
            pzw = psw.tile([P, D], F32, tag="pw", name=f"pzw{m}")
            for kc in range(2):
                nc.tensor.matmul(
                    pzw[:], lhsT=zb[:, kc * D + m * P:kc * D + (m + 1) * P],
                    rhs=Wb[:, kc * D:(kc + 1) * D],
                    start=(kc == 0), stop=(kc == 1),
                )
            nc.vector.tensor_copy(zwb[:, m * D:(m + 1) * D], pzw[:])

        # ---- phase C: out = f * (x_local @ zw) ----
        for i in range(LT):
            po = psw.tile([P, D], F32, tag="pw", name=f"po{i}")
            for c in range(2):
                nc.tensor.matmul(
                    po[:], lhsT=xT[:, c * NL + i * P:c * NL + (i + 1) * P],
                    rhs=zwb[:, c * D:(c + 1) * D],
                    start=(c == 0), stop=(c == 1),
                )
            o_sb = wp.tile([P, D], F32, tag="o", name=f"o{i}")
            nc.scalar.mul(o_sb[:], po[:], f_t[:, i:i + 1])
            nc.sync.dma_start(out[i * P:(i + 1) * P, :], o_sb[:])


def _build():
    nc = bacc.Bacc("TRN2", target_bir_lowering=False, debug=False, num_devices=R)
    x = nc.dram_tensor("x", [N, D], BF16, kind="ExternalInput")
    xTl = nc.dram_tensor("xTl", [D, NL], BF16, kind="ExternalInput")
    W = nc.dram_tensor("W", [D, D], F32, kind="ExternalInput")
    out = nc.dram_tensor("out", [NL, D], F32, kind="ExternalOutput")
    with tile.TileContext(nc) as tc:
        _program(tc,
                 x.ap() if hasattr(x, "ap") else x,
                 xTl.ap() if hasattr(xTl, "ap") else xTl,
                 W.ap() if hasattr(W, "ap") else W,
                 out.ap() if hasattr(out, "ap") else out)
    nc.finalize()
    return nc


def _run(inputs, trace=False):
    if "nc" not in _cache:
        _cache["nc"] = _build()
    nc = _cache["nc"]
    x = np.asarray(inputs["x"], dtype=np.float32)
    W = np.ascontiguousarray(inputs["W"], dtype=np.float32)
    xb16 = x.astype(bfloat16)
    in_maps = []
    for r in range(R):
        # roll so core r's own 1024 rows come first -> rank-free program
        xr = np.ascontiguousarray(
            np.concatenate([xb16[r * NL:], xb16[:r * NL]], axis=0))
        xTl = np.ascontiguousarray(xb16[r * NL:(r + 1) * NL].T)
        in_maps.append({"x": xr, "xTl": xTl, "W": W})
    res = bass_utils.run_bass_kernel_spmd(
        nc, in_maps, core_ids=list(range(R)), trace=trace,
    )
    out = np.concatenate([res.results[r]["out"] for r in range(R)], axis=0)
    return out, res


def kernel(**inputs) -> np.ndarray:
    out, _ = _run(inputs, trace=False)
    return out
